# revision 2
# baseline (speedup 1.0000x reference)
"""GAT layer (nn_GATLayerAdj) Trainium2 Bass kernel, 8-core SPMD.

Reference computation (N=1024, di=do=64):
    a[i,j]  = x[j]@w_src + x[i]@w_tgt + bw        (attention logits)
    att     = softmax_j(where(adj>0, a, -1e16))
    y[i,j,:]= relu(x[j]@WfS.T + x[i]@WfT.T + bf)
    o[i,:]  = sum_j att[i,j] * y[i,j,:]

Key factorization: e[i,j] = exp(a[i,j])*M[i,j] with M = (adj>0) splits as
exp(atgt[i]+bw) * exp(asrc[j]) * M[i,j]; the row factor cancels in the
softmax, so att[i,j] = g[j]M[i,j] / sum_j g[j]M[i,j] with g = exp(asrc).
The device therefore needs NO exp / softmax / transposes at all: the host
uploads e'^T[j,i] = g[j]*M[i,j] (transposed, PE-stationary-ready, same
O(N^2) prep class as the old adjm mask), and the denominator s'[i] =
sum_j e'^T[j,i] is 8 single-column matmuls on the otherwise idle PE.

Sharding: target-node dim i split across 8 cores (128 target rows each).

Host prep is otherwise as before: ys = x@WfS.T, u = x@WfT.T + bf in
numpy; all O(N^2 d) work runs on device.

Per-core schedule (source dim j on partitions):
  1. DMAs orderd by need-time: ysjp, urep g0/g1 (first adds), etp
     (s' + first reduce), urep g2..g7. urep[j,(i,d)] = u broadcast
     across partitions via stride-0 DMA reads (2MB).
  2. s' PSUM accumulation (8 matmuls, N=1) -> reciprocal -> r_t.
  3. HALF-major build: pass h processes free columns [4096h, 4096h+4096)
     of ALL chunks. Z = ys_bcast + urep on DVE (tensor_tensor, 2x bf16);
     relu split DVE (tensor_scalar_max, 4x) / ACT per a balance table.
  4. T_acc[i',(i,d)] += e'^T chunk matmuls, col groups b=2h,2h+1 per
     pass, 4x32 PSUM partitions. On the final (c,h=1) slice the four
     n2 evacuations (scale=1/s', ACT/DVE alternating) stream out behind
     the PE with their output DMAs; host gathers the 32-wide diagonal.

Numerics: bf16 inputs to the adds/matmuls, fp32 accumulation, bf16
output (host upcasts).
"""

from contextlib import ExitStack

import numpy as np
import ml_dtypes

import concourse.bass as bass
import concourse.tile as tile
from concourse import bacc, mybir
from concourse.bass_utils import run_bass_kernel_spmd

# Lighter TileContext exit: stock emits drain + full butterfly barrier +
# sem clears + second butterfly (~11us). Engines already sync at program
# end; keep the drain (output DMA completion), a sem-only rendezvous
# before the clears, and drop the trailing barrier.
import concourse.tile as _tile_mod

if not getattr(_tile_mod, "_exit_trimmed", False):
    def _drain_and_barrier_trim(self, tick_clock, wait_clock):
        from concourse.tile import ScopedClock
        nc = self.nc
        drain_inst = nc.sync.drain()
        wait_clock.add_sem_waits(
            drain_inst.ins, ScopedClock({None: tick_clock.global_clock})
        )
        exit_sem = nc.alloc_semaphore("exit_rdv")
        for eng in (nc.sync, nc.tensor, nc.vector, nc.scalar):
            eng.nop(nofuse=True).then_inc(exit_sem, 1)
        nc.gpsimd.wait_ge(exit_sem, 4)
        assert self.sems is not None
        popped = nc._tile_sem_poison_stack.pop()
        assert popped is self._sem_poison
        nc.clear_and_free_semaphores(list(self.sems.allocated().values()))
        nc.gpsimd.sem_clear(range(exit_sem.num, exit_sem.num + 1))

    _tile_mod.TileContext._drain_and_barrier = _drain_and_barrier_trim
    _tile_mod._exit_trimmed = True

N = 1024
DI = 64
DO = 64
N_CORES = 8
ROWS = N // N_CORES          # 128 target rows per core
NCHUNK = N // 128            # 8 j-chunks
F_FULL = ROWS * DO           # 8192 free size of (i, d)
HALF = F_FULL // 2           # 4096
QUART = F_FULL // 4          # 2048

f32 = mybir.dt.float32
bf16 = mybir.dt.bfloat16
AF = mybir.ActivationFunctionType
ALU = mybir.AluOpType

# etp blob columns: [e'^T 1024 | ones 8]
E_W = N + 8

# Per-pass chunk emission order and relu-engine split. 'A' = one ACT
# [128,4096] relu, 'D' = two DVE tensor_scalar_max [128,2048] (4x).
# A-chunk TTs must arrive at ACT every <=3.7us or ACT starves, so D
# chunks are interleaved between A chunks (TT 2.28 + D-relu 1.36 =
# 3.64us of DVE work per A-TT gap). Balance: DVE = 16x2.28 + 5x1.36
# ~= 43; ACT = 11x3.69 ~= 41us.
PASS_ORDER = [
    [(0, "A"), (1, "D"), (2, "A"), (4, "D"), (3, "A"), (6, "D"),
     (5, "A"), (7, "A")],
    [(2, "A"), (0, "D"), (3, "A"), (1, "D"), (4, "A"), (5, "D"),
     (6, "A"), (7, "A")],
]

_CACHE = {}


def _build_program():
    nc = bacc.Bacc("TRN2", target_bir_lowering=False, debug=False,
                   num_devices=N_CORES)

    # ---- DRAM I/O ----
    ysjp_d = nc.dram_tensor("ysjp", [128, NCHUNK * DO], bf16,
                            kind="ExternalInput").ap()
    etp_d = nc.dram_tensor("etp", [128, E_W], bf16,
                           kind="ExternalInput").ap()
    uflat_d = nc.dram_tensor("uflat", [F_FULL], bf16, kind="ExternalInput").ap()
    o_d = nc.dram_tensor("o", [128, 2048], bf16, kind="ExternalOutput").ap()

    with tile.TileContext(nc) as tc, ExitStack() as ctx:
        cons = ctx.enter_context(tc.tile_pool(name="cons", bufs=1))
        zp = ctx.enter_context(tc.tile_pool(name="zp", bufs=3))
        rp = ctx.enter_context(tc.tile_pool(name="rp", bufs=3))
        psp = ctx.enter_context(tc.tile_pool(name="psp", bufs=2, space="PSUM"))
        accp = ctx.enter_context(tc.tile_pool(name="accp", bufs=1, space="PSUM"))

        # ---- DMAs, all on the sync queue (HWDGE: packets flow ~2us
        # earlier than SWDGE). Order by need-time: ysjp (first adds),
        # urep g0+g1 (first build pass), etp (s' sums + reduce
        # stationary), then g2..g7.
        ys_jp = cons.tile([128, NCHUNK * DO], bf16)
        nc.sync.dma_start(ys_jp[:], ysjp_d[:, :])
        urep = cons.tile([128, F_FULL], bf16)
        etp = cons.tile([128, E_W], bf16)

        EIGHTH = F_FULL // 8

        def bcast(g):
            sl = slice(EIGHTH * g, EIGHTH * (g + 1))
            src = uflat_d[sl]
            bsrc = bass.AP(tensor=src.tensor, offset=src.offset,
                           ap=[[0, 128]] + [list(d) for d in src.ap])
            nc.sync.dma_start(out=urep[:, sl], in_=bsrc)

        bcast(0)
        bcast(1)
        nc.sync.dma_start(etp[:], etp_d[:, :])
        for g in range(2, 8):
            bcast(g)
        et_all = etp[:, 0:N]
        ones_t = etp[:, N:N + 1]

        r_t = cons.tile([ROWS, 1], f32)
        # one PSUM tile per 512-col bank so the final evacuations do not
        # create false tile-level dependencies against remaining matmuls
        t_accs = [accp.tile([128, 512], f32, tag=f"acc{n2}", name=f"t_acc{n2}")
                  for n2 in range(4)]
        t_sb = cons.tile([128, 2048], bf16)

        def emit_denom():
            # s'[i] = sum_j e'^T[j,i]: 8 accumulating single-column
            # matmuls on the otherwise-idle PE, then 1/s' on DVE.
            s_ps = psp.tile([128, 1], f32, tag="sps", name="s_ps")
            for c in range(NCHUNK):
                nc.tensor.matmul(s_ps[:], et_all[:, 128 * c:128 * (c + 1)],
                                 ones_t[:], start=(c == 0),
                                 stop=(c == NCHUNK - 1),
                                 skip_group_check=True)
            nc.vector.reciprocal(r_t[:], s_ps[:])

        def emit_build(h, c, eng):
            r_c = rp.tile([128, HALF], bf16, name="r_c")
            z = zp.tile([128, HALF], bf16, name="z")
            ys_c = ys_jp[:, DO * c:DO * (c + 1)]
            # chunk 0 of pass 0: four 1024-wide sub-adds so the first one
            # only waits for the first 256KB broadcast slice
            nsub = 4 if (h, c) == (0, 0) else 1
            step = HALF // nsub
            for sb in range(nsub):
                sl = slice(HALF * h + step * sb, HALF * h + step * (sb + 1))
                zl = slice(step * sb, step * (sb + 1))
                ys_b = ys_c.rearrange("p d -> p () d").broadcast_to(
                    (128, step // DO, DO))
                zv = z[:, zl].rearrange("p (i d) -> p i d", i=step // DO)
                uv = urep[:, sl].rearrange("p (i d) -> p i d", i=step // DO)
                nc.vector.tensor_tensor(zv, ys_b, uv, ALU.add)
            if eng == "D":
                nc.vector.tensor_scalar_max(r_c[:, 0:QUART], z[:, 0:QUART], 0.0)
                nc.vector.tensor_scalar_max(r_c[:, QUART:], z[:, QUART:], 0.0)
            else:
                nc.scalar.activation(r_c[:], z[:], AF.Relu)
            return r_c

        def emit_reduce(h, c, r_c, first, last):
            for bq in range(2):
                b = 2 * h + bq
                for n2 in range(4):
                    nc.tensor.matmul(
                        t_accs[n2][32 * b:32 * (b + 1), :],
                        et_all[:, 128 * c + 32 * b:128 * c + 32 * (b + 1)],
                        r_c[:, 2048 * bq + 512 * n2:2048 * bq + 512 * (n2 + 1)],
                        start=first,
                        stop=last,
                        skip_group_check=True,
                        tile_position=(0, 32 * b),
                    )
            if last and h == 1:
                # all accumulation done: four scaled evacuations
                # (scale=1/s', DVE/ACT alternating) + output DMAs; each
                # t_accs[n2] is its own PSUM tile so these do not
                # serialize against the preceding matmuls of other banks
                for n2 in range(4):
                    osl = slice(512 * n2, 512 * (n2 + 1))
                    if n2 % 2 == 0:
                        nc.vector.tensor_scalar_mul(t_sb[:, osl],
                                                    t_accs[n2][:, :], r_t[:])
                    else:
                        nc.scalar.activation(t_sb[:, osl], t_accs[n2][:, :],
                                             AF.Copy, bias=0.0,
                                             scale=r_t[:])
                    nc.sync.dma_start(out=o_d[:, osl], in_=t_sb[:, osl])

        emit_denom()
        for h in range(2):
            order = PASS_ORDER[h]
            pend = None
            for k in range(len(order) + 1):
                if k < len(order):
                    c, eng = order[k]
                    built = (c, emit_build(h, c, eng))
                if k >= 1:
                    pc, pr = pend
                    emit_reduce(h, pc, pr, first=(k == 1),
                                last=(k == len(order)))
                pend = built

    nc.compile()
    return nc


def _prep_inputs(x, adj, Wf, bf_, Ww, bw):
    b = ml_dtypes.bfloat16
    x64 = x.astype(np.float64)
    ys = (x64 @ Wf[:, :DI].astype(np.float64).T).astype(np.float32)   # [N, 64]
    u = (x64 @ Wf[:, DI:].astype(np.float64).T + bf_).astype(np.float32)
    asrc = (x64 @ Ww[0, :DI].astype(np.float64)).astype(np.float32)   # [N]
    g = np.exp(asrc.astype(np.float64)).astype(np.float32)            # [N]

    # ysjp[jl, 64c+d] = ys[128c+jl, d]
    ysjp = ys.reshape(NCHUNK, 128, DO).transpose(1, 0, 2).reshape(128, -1)
    # e'^T[j, i] = g[j] * (adj[i, j] > 0), chunk-packed like et_all:
    # etp[jl, 128c+il] = e'^T[128c+jl, il]
    mask_t = (adj > 0).T.astype(np.float32)          # [j, i]

    in_maps = []
    for c in range(N_CORES):
        blk = slice(ROWS * c, ROWS * (c + 1))
        et = mask_t[:, blk] * g[:, None]              # [1024, 128]
        etp = np.zeros((128, E_W), np.float32)
        etp[:, 0:N] = et.reshape(NCHUNK, 128, ROWS).transpose(1, 0, 2).reshape(128, -1)
        etp[:, N:N + 1] = 1.0
        m = dict(
            ysjp=np.ascontiguousarray(ysjp).astype(b),
            etp=etp.astype(b),
            uflat=np.ascontiguousarray(u[blk].reshape(F_FULL)).astype(b),
        )
        in_maps.append(m)
    return in_maps


def get_program():
    if "nc" not in _CACHE:
        _CACHE["nc"] = _build_program()
    return _CACHE["nc"]


def unpack_output(res_list):
    p_idx = np.arange(128)
    col0 = (p_idx % 32) * DO
    cols = col0[:, None] + np.arange(DO)[None, :]
    out = np.empty((N, DO), np.float32)
    for c in range(N_CORES):
        t = res_list[c]["o"].astype(np.float32)      # [128, 2048]
        out[ROWS * c:ROWS * (c + 1)] = t[p_idx[:, None], cols]
    return out


def kernel(x, adj, Wf, bf, Ww, bw):
    x = np.asarray(x, dtype=np.float32)
    adj = np.asarray(adj, dtype=np.int32)
    Wf = np.asarray(Wf, dtype=np.float32)
    bf_ = np.asarray(bf, dtype=np.float32)
    Ww = np.asarray(Ww, dtype=np.float32)
    bw = np.asarray(bw, dtype=np.float32)
    assert x.shape == (N, DI) and adj.shape == (N, N)

    nc = get_program()
    in_maps = _prep_inputs(x, adj, Wf, bf_, Ww, bw)
    res = run_bass_kernel_spmd(nc, in_maps, core_ids=list(range(N_CORES)))
    return unpack_output(res.results)
